# revision 50
# baseline (speedup 1.0000x reference)
"""Trainium2 Bass kernel for nn_Attention_77043123355775.

Sharded GQA causal attention with RoPE: 8 NeuronCores as 2-way data
parallel (batch) x 4-way tensor parallel (heads). Each core computes its
4 Q heads / 2 KV heads for one batch entry and a partial output
projection (x[b] @ W)^T; the host sums the 4 partials per batch.

All matmuls run in plain bf16 with fp32 PSUM accumulation (the 2e-2
rel-err budget has ~20x headroom over bf16 rounding noise). Weights are
fully SBUF-resident. The PE instruction stream interleaves next-chunk
QKV/V projection chains and prev-chunk output-projection groups between
attention blocks, so the PE never waits on the scalar-engine exp chain
and stays in its high DVFS p-state.
"""
import math
import os
import sys

for _p in ("/opt/trn_rl_repo",):
    if _p not in sys.path:
        sys.path.insert(0, _p)

import ml_dtypes
import numpy as np

import concourse.bass as bass
import concourse.mybir as mybir
import concourse.tile as tile

from concourse.tile import add_dep_helper

dt = mybir.dt
AF = mybir.ActivationFunctionType


def build_attention_nc(S=2048, D=2048, NQ=4, NKV=2, HD=128, TC=512):
    assert HD == 128
    C = D // 128          # contraction chunks over features
    TB = S // 128         # 128-token blocks
    NTC = S // TC         # token chunks
    DB = D // 128         # output feature blocks
    CO = NQ * HD // 128   # contraction chunks for wo (= NQ)
    REP = NQ // NKV
    CQ = C // 4           # c-chunks per x quarter-tile
    NTB = TC // 128       # token blocks per chunk
    scale = 1.0 / math.sqrt(HD)

    nc = bass.Bass()

    xt = nc.dram_tensor("xt", [D, S], dt.bfloat16, kind="ExternalInput")
    ident = nc.dram_tensor("ident", [128, 128], dt.bfloat16, kind="ExternalInput")
    wqp = nc.dram_tensor("wqp", [D, NQ * HD], dt.bfloat16, kind="ExternalInput")
    wkp = nc.dram_tensor("wkp", [D, NKV * HD], dt.bfloat16, kind="ExternalInput")
    wvp = nc.dram_tensor("wvp", [D, NKV * HD], dt.bfloat16, kind="ExternalInput")
    woh = nc.dram_tensor("woh", [NQ * HD, D], dt.bfloat16, kind="ExternalInput")
    csT = nc.dram_tensor("csT", [HD, S], dt.bfloat16, kind="ExternalInput")
    masks = nc.dram_tensor("masks", [4 * 128, TC], dt.bfloat16, kind="ExternalInput")
    outT = nc.dram_tensor("outT", [D, S], dt.bfloat16, kind="ExternalOutput")

    with tile.TileContext(nc) as tc:
        with (
            tc.tile_pool(name="const", bufs=1) as constp,
            tc.tile_pool(name="tabs", bufs=1) as tabp,
            tc.tile_pool(name="wts", bufs=1) as wtp,
            tc.tile_pool(name="acts", bufs=1) as actp,
            tc.tile_pool(name="chunkacts", bufs=1) as cap,
            tc.tile_pool(name="xstream", bufs=8) as xsp,
            tc.tile_pool(name="scratch", bufs=3) as scr,
            tc.tile_pool(name="psum", bufs=1, space="PSUM") as psp,
        ):
            ident_t = constp.tile([128, 128], dt.bfloat16, tag="ident")

            # ---- resident tables / weights (c-quartered for fine deps) ----
            def emit_w_dma(tiles, src, g, ring):
                cq = C // len(tiles)
                rs = slice(g * cq * 128, (g + 1) * cq * 128)
                return ring.dma_start(
                    tiles[g].rearrange("p (c n) -> p c n", c=cq),
                    src[rs, :].rearrange("(c p) n -> p c n", p=128),
                )

            wq_ts = [wtp.tile([128, CQ * NQ * HD], dt.bfloat16, tag=f"wq{g}", name=f"wq{g}") for g in range(4)]
            wk_ts = [wtp.tile([128, (C // 2) * NKV * HD], dt.bfloat16, tag=f"wk{g}", name=f"wk{g}") for g in range(2)]
            wv_ts = [wtp.tile([128, (C // 2) * NKV * HD], dt.bfloat16, tag=f"wv{g}", name=f"wv{g}") for g in range(2)]
            wo_t = wtp.tile([128, CO * D], dt.bfloat16, tag="wo")

            def wq_sl(c, h):
                return wq_ts[c // CQ][:, (c % CQ) * NQ * HD + h * HD:(c % CQ) * NQ * HD + (h + 1) * HD]

            def wk_sl(c, h):
                ch = C // 2
                return wk_ts[c // ch][:, (c % ch) * NKV * HD + h * HD:(c % ch) * NKV * HD + (h + 1) * HD]

            def wv_sl(c):
                ch = C // 2
                return wv_ts[c // ch][:, (c % ch) * NKV * HD:(c % ch + 1) * NKV * HD]

            cs_t = tabp.tile([HD, S], dt.bfloat16, tag="cs")
            cos_t = cs_t[0:HD // 2, :]
            sin_t = cs_t[HD // 2:HD, :]
            mask_t = [tabp.tile([128, TC], dt.bfloat16, tag=f"mask{i}", name=f"mask{i}") for i in range(4)]

            xq_tiles = {}

            def emit_x_dmas(tci, rings):
                ts_ = slice(tci * TC, (tci + 1) * TC)
                tiles = []
                for g in range(4):
                    rs = slice(g * CQ * 128, (g + 1) * CQ * 128)
                    t = xsp.tile([128, CQ * TC], dt.bfloat16, tag="xq",
                                 name=f"x_{tci}_{g}")
                    rings[g].dma_start(
                        t.rearrange("p (c n) -> p c n", c=CQ),
                        xt[rs, ts_].rearrange("(c p) n -> p c n", p=128),
                    )
                    tiles.append(t)
                xq_tiles[tci] = tiles

            # startup: the first QKV chain consumes (wq_g, x_g) pairs in
            # quarter order, so interleave them pairwise on the fast sync
            # ring; gpsimd carries the RoPE tables + K/V weights (needed
            # a few microseconds later).
            x0_tiles = []
            for g in range(4):
                emit_w_dma(wq_ts, wqp, g, nc.sync)
                rs = slice(g * CQ * 128, (g + 1) * CQ * 128)
                t = xsp.tile([128, CQ * TC], dt.bfloat16, tag="xq", name=f"x_0_{g}")
                nc.sync.dma_start(
                    t.rearrange("p (c n) -> p c n", c=CQ),
                    xt[rs, 0:TC].rearrange("(c p) n -> p c n", p=128),
                )
                x0_tiles.append(t)
            xq_tiles[0] = x0_tiles
            # The secondary loads (cs/wk/wv/masks) would otherwise compete
            # with the critical wq/x burst for chip HBM bandwidth at t=0
            # (all 8 cores slurp simultaneously); throttle them behind
            # early Q-chain matmuls via explicit deps filled in later.
            startup_dmas = []
            startup_dmas.append(nc.gpsimd.dma_start(cs_t[:], csT[:]))
            startup_dmas.append(emit_w_dma(wk_ts, wkp, 0, nc.gpsimd))
            startup_dmas.append(emit_w_dma(wk_ts, wkp, 1, nc.gpsimd))
            startup_dmas.append(emit_w_dma(wv_ts, wvp, 0, nc.gpsimd))
            startup_dmas.append(emit_w_dma(wv_ts, wvp, 1, nc.gpsimd))
            for i in range(4):
                nc.gpsimd.dma_start(mask_t[i][:], masks[i * 128:(i + 1) * 128, :])
            nc.gpsimd.dma_start(ident_t[:], ident[:])
            nc.sync.dma_start(
                wo_t.rearrange("p (c n) -> p c n", c=CO),
                woh.rearrange("(c p) n -> p c n", p=128),
            )
            startup_mms = []

            # K/V persist per 512-chunk / 128-block (no cross-chunk tiles,
            # so interleaved next-chunk RoPE writes never alias attention
            # reads at the dep tracker's granularity)
            ktc = [[actp.tile([128, TC], dt.bfloat16, tag=f"kt{h}_{j}", name=f"kt{h}_{j}")
                    for j in range(NTC)] for h in range(NKV)]
            # V tiles carry a ones column per kv head (col kv*(HD+1)+HD) so
            # the flipped PV matmul emits softmax denominators for free
            vt = [actp.tile([128, NKV * (HD + 1)], dt.bfloat16, tag=f"vt{b}", name=f"vt{b}") for b in range(TB)]
            for b in range(TB):
                for kv in range(NKV):
                    nc.vector.memset(vt[b][:, kv * (HD + 1) + HD:(kv + 1) * (HD + 1)], 1.0)
            qt_all = {}
            ot_all = {}
            for tci in range(NTC):
                qt_all[tci] = [cap.tile([128, TC], dt.bfloat16, tag=f"qt{h}_{tci % 2}", name=f"qt{h}_{tci}") for h in range(NQ)]
                ot_all[tci] = [cap.tile([128, TC], dt.bfloat16, tag=f"ot{h}_{tci % 2}", name=f"ot{h}_{tci}") for h in range(NQ)]

            # ---------------- unit generators ----------------
            def rope_epilogue(tci, h, ps):
                ts_ = slice(tci * TC, (tci + 1) * TC)
                rot = scr.tile([128, TC], dt.bfloat16, tag="rope", bufs=2)
                t0 = scr.tile([128, TC], dt.bfloat16, tag="ropetmp", bufs=1)
                cs = cos_t[:, ts_]
                sn = sin_t[:, ts_]
                xr = ps[0:64, :]
                xi = ps[64:128, :]
                dsth = qt_all[tci][h][:] if h < NQ else ktc[h - NQ][tci][:]
                nc.vector.tensor_tensor(rot[0:64, :], xr, cs, mybir.AluOpType.mult)
                nc.vector.tensor_tensor(t0[0:64, :], xi, sn, mybir.AluOpType.mult)
                nc.vector.tensor_tensor(dsth[0:64, :], rot[0:64, :], t0[0:64, :], mybir.AluOpType.subtract)
                nc.vector.tensor_tensor(rot[64:128, :], xr, sn, mybir.AluOpType.mult)
                nc.vector.tensor_tensor(t0[64:128, :], xi, cs, mybir.AluOpType.mult)
                nc.vector.tensor_tensor(dsth[64:128, :], rot[64:128, :], t0[64:128, :], mybir.AluOpType.add)

            def qkv_units(tci, ptag, pbufs, group=4, tagmap=None):
                """Closures emitting `group` matmuls of a QKV/V chain each
                (coarser units keep chain psum lifetimes short). `tagmap`
                optionally assigns (tag, bufs) per chain so chunk-0's
                wavefront can borrow the attention-phase psum banks that
                are idle during startup."""
                units = []
                state = {}

                def pick(key):
                    if tagmap and key in tagmap:
                        return tagmap[key]
                    return ptag, pbufs

                def x_c(c):
                    xg = xq_tiles[tci]
                    return xg[c // CQ][:, (c % CQ) * TC:(c % CQ + 1) * TC]

                def qk_seg(h, c0):
                    if c0 == 0:
                        tg, bf = pick(h)
                        state[h] = psp.tile([128, TC], dt.float32, tag=tg,
                                            bufs=bf, name=f"qkv_{tci}_{h}")
                    ps = state[h]
                    for c in range(c0, c0 + group):
                        wsl = wq_sl(c, h) if h < NQ else wk_sl(c, h - NQ)
                        mm = nc.tensor.matmul(ps[:], wsl, x_c(c),
                                              start=(c == 0), stop=(c == C - 1))
                        if tci == 0 and h < 2:
                            startup_mms.append(mm)
                    if c0 + group == C:
                        rope_epilogue(tci, h, ps)

                def v_seg(tb, c0):
                    key = "v", tb
                    if c0 == 0:
                        tg, bf = pick(key)
                        state[key] = psp.tile([128, NKV * HD], dt.float32,
                                              tag=tg, bufs=bf,
                                              name=f"v_{tci}_{tb}")
                    ps = state[key]
                    for c in range(c0, c0 + group):
                        nc.tensor.matmul(ps[:], x_c(c)[:, tb * 128:(tb + 1) * 128],
                                         wv_sl(c),
                                         start=(c == 0), stop=(c == C - 1))
                    if c0 + group == C:
                        for kv in range(NKV):
                            nc.scalar.copy(
                                vt[tci * NTB + tb][:, kv * (HD + 1):kv * (HD + 1) + HD],
                                ps[:, kv * HD:(kv + 1) * HD])

                for h in range(NQ + NKV):
                    for c0 in range(0, C, group):
                        units.append(lambda h=h, c0=c0: qk_seg(h, c0))
                for tb in range(NTB):
                    for c0 in range(0, C, group):
                        units.append(lambda tb=tb, c0=c0: v_seg(tb, c0))
                return units

            def op_units(tci, ptag, pbufs):
                """One closure per output-projection db group (4 matmuls +
                copy + store)."""
                ts_ = slice(tci * TC, (tci + 1) * TC)
                ot = ot_all[tci]
                units = []
                for db in range(DB):
                    def u(db=db):
                        ps = psp.tile([128, TC], dt.float32, tag=ptag, bufs=pbufs,
                                      name=f"op_{tci}_{db}")
                        for c in range(CO):
                            nc.tensor.matmul(
                                ps[:], wo_t[:, c * D + db * 128:c * D + (db + 1) * 128],
                                ot[c][:],
                                start=(c == 0), stop=(c == CO - 1),
                            )
                        o3 = scr.tile([128, TC], dt.bfloat16, tag="o3", bufs=8)
                        if tci == NTC - 1 and db % 2 == 1:
                            nc.vector.tensor_copy(o3[:], ps[:])
                        else:
                            nc.scalar.copy(o3[:], ps[:])
                        eng = nc.sync if db % 2 == 0 else nc.gpsimd
                        eng.dma_start(outT[db * 128:(db + 1) * 128, ts_], o3[:])
                    units.append(u)
                return units

            # ---------------- attention emission ----------------
            def emit_attention(tci, filler):
                """Scores + flipped PV for q-chunk tci, draining `filler`
                closures between steps to keep the PE busy. PV runs
                qsb-major: each (head, q-subtile) accumulates [q,HD+1]
                (output + denominator column) as a single group in its own
                PSUM bank -- interleaved groups in one bank corrupt."""
                qc = tci
                qt = qt_all[tci]
                ot = ot_all[tci]
                nkb = (qc + 1) * NTB
                nf = len(filler)
                total_steps = (2 * NQ + 1) * nkb
                state = {"step": 0, "drained": 0}

                def drain():
                    state["step"] += 1
                    want = state["step"] * nf // total_steps
                    while state["drained"] < want:
                        filler[state["drained"]]()
                        state["drained"] += 1

                def emit_scores(h, kb):
                    kv = h // REP
                    d = kb * 128 - qc * TC
                    q0 = max(d, 0)
                    sc_ps = psp.tile([128, TC], dt.float32, tag="sc", bufs=3,
                                     name=f"sc_{tci}_{h}_{kb}")
                    ksl = ktc[kv][kb // NTB][:, (kb % NTB) * 128:(kb % NTB + 1) * 128]
                    nc.tensor.matmul(sc_ps[:, q0:TC], ksl, qt[h][:, q0:TC],
                                     start=True, stop=True)
                    ph = scr.tile([128, TC], dt.bfloat16, tag="ph", bufs=36,
                                  name=f"ph_{tci}_{h}_{kb}")
                    nc.scalar.activation(ph[:, q0:TC], sc_ps[:, q0:TC], AF.Exp,
                                         bias=0.0, scale=scale)
                    if d >= 0:
                        nc.vector.tensor_tensor(ph[:, q0:TC], ph[:, q0:TC],
                                                mask_t[d // 128][:, q0:TC],
                                                mybir.AluOpType.mult)
                    return ph

                def norm_one(h, qsb, t_):
                    rec = scr.tile([128, 1], dt.float32, tag="recq", bufs=4,
                                   name=f"rec_{tci}_{h}_{qsb}")
                    nc.vector.reciprocal(rec[:], t_[:, HD:HD + 1])
                    otn = scr.tile([128, HD], dt.bfloat16, tag="otn", bufs=4,
                                   name=f"otn_{tci}_{h}_{qsb}")
                    nc.vector.tensor_scalar(otn[:], t_[:, 0:HD], rec[:], None,
                                            mybir.AluOpType.mult)
                    tp = psp.tile([128, 128], dt.bfloat16, tag="sc", bufs=3,
                                  name=f"tp_{tci}_{h}_{qsb}")
                    nc.tensor.transpose(tp[:], otn[:], ident_t[:])
                    nc.scalar.copy(ot[h][:, qsb * 128:(qsb + 1) * 128], tp[:])

                # head software pipeline: scores of head h+1 are emitted
                # between PV chain steps of head h, so the scalar exp chain
                # for the next head runs under the current head's PE work
                all_phs = {}

                def score_units(h):
                    def u(h=h, kb=None):
                        pass
                    us = []
                    for kb in range(nkb):
                        def uu(h=h, kb=kb):
                            all_phs.setdefault(h, []).append(emit_scores(h, kb))
                        us.append(uu)
                    return us

                def chains_for(h, next_scores):
                    kv = h // REP
                    phs = all_phs[h]
                    nsteps = 2 * nkb
                    si = 0
                    step = 0
                    for qsb0 in (0, 2):
                        chains = []
                        for qsb in (qsb0, qsb0 + 1):
                            t_ = psp.tile([128, HD + 1], dt.float32, tag="otq",
                                          bufs=2, name=f"otq_{tci}_{h}_{qsb}")
                            chains.append((qsb, t_))
                        for kb in range(nkb):
                            d = kb * 128 - qc * TC
                            q0 = max(d, 0)
                            vsl = vt[kb][:, kv * (HD + 1):(kv + 1) * (HD + 1)]
                            for qsb, t_ in chains:
                                if kb > qc * NTB + qsb or qsb < q0 // 128:
                                    continue
                                nc.tensor.matmul(
                                    t_[:], phs[kb][:, qsb * 128:(qsb + 1) * 128], vsl,
                                    start=(kb == 0), stop=(kb == qc * NTB + qsb),
                                )
                            step += 1
                            want_s = step * len(next_scores) // nsteps
                            while si < want_s:
                                next_scores[si]()
                                si += 1
                            drain()
                        for qsb, t_ in chains:
                            norm_one(h, qsb, t_)
                    while si < len(next_scores):
                        next_scores[si]()
                        si += 1

                for u in score_units(0):
                    u()
                    drain()
                for h in range(NQ):
                    chains_for(h, score_units(h + 1) if h + 1 < NQ else [])
                while state["drained"] < nf:
                    filler[state["drained"]]()
                    state["drained"] += 1

            # ---------------- schedule ----------------
            # QKV0 standalone; attn(t) interleaves QKV(t+1) and outproj(t-1);
            # outproj(NTC-1) standalone.
            # chunk-0 wavefront: Q0..Q2 advance quarter-by-quarter as x/wq
            # land (3 concurrent chains = the 3 "sc" banks); later chains
            # run after their data has arrived
            _tag0 = {0: ("sc", 3), 1: ("sc", 3), 2: ("sc", 3),
                     3: ("fill", 2), 4: ("fill", 2), 5: ("op", 1),
                     ("v", 0): ("otq", 2), ("v", 1): ("otq", 2),
                     ("v", 2): ("sc", 3), ("v", 3): ("fill", 2)}
            _u0 = qkv_units(0, "sc", 3, tagmap=_tag0)
            _ng = C // 4
            _wave = []
            for g in range(_ng):
                for h in range(3):
                    _wave.append(_u0[h * _ng + g])
            for i in range(3 * _ng, len(_u0)):
                _wave.append(_u0[i])
            for u in _wave:
                u()
            # cs waits Q0-chain c2; wk halves wait Q0 end / Q1 mid; wv later
            _anchors = [2, 14, 18, 22, 26]
            for _d, _a in zip(startup_dmas, _anchors):
                if _d is not None and _a < len(startup_mms):
                    add_dep_helper(_d.ins, startup_mms[_a].ins,
                                   reason="startup HBM burst throttle")
            for tci in range(NTC):
                if tci + 1 < NTC:
                    emit_x_dmas(tci + 1, [nc.sync, nc.sync, nc.gpsimd, nc.gpsimd])
                filler = []
                if tci > 0:
                    filler += op_units(tci - 1, "op", 1)
                if tci + 1 < NTC:
                    filler += qkv_units(tci + 1, "fill", 2)
                emit_attention(tci, filler)
                xq_tiles.pop(tci)
            for u in op_units(NTC - 1, "sc", 3):
                u()

    return nc


# ---------------------------------------------------------------------------
# walrus in this container refuses >1 sem wait per instruction ("Too many
# sync wait commands"). Hoist excess waits onto same-engine NoOps inserted
# immediately before the instruction - program order on the engine queue
# preserves the sync semantics.
def split_multiwait_insts(nc, max_waits=1):
    n_split = 0
    for bb in nc.main_func.blocks:
        insts = bb.instructions
        i = 0
        while i < len(insts):
            ins = insts[i]
            si = getattr(ins, "sync_info", None)
            if si is not None and si.on_wait and len(si.on_wait) > max_waits:
                waits = list(si.on_wait)
                head, tail = waits[:-max_waits], waits[-max_waits:]
                nops = []
                for j in range(0, len(head), max_waits):
                    nop = mybir.InstNoOp(name=f"{ins.name}-ws{j}", ins=[], outs=[])
                    nop.engine = ins.engine
                    nop.sync_info = mybir.SyncInfo(
                        on_wait=head[j:j + max_waits], on_update=[])
                    nops.append(nop)
                ins.sync_info = mybir.SyncInfo(
                    on_wait=tail, on_update=list(si.on_update or []))
                insts[i:i] = nops
                i += len(nops)
                n_split += 1
            i += 1
    return n_split


# ---------------------------------------------------------------------------
# Host-side shard preparation / gather
BF16 = ml_dtypes.bfloat16


def rope_tables(S, HD):
    inv = 1.0 / (10000.0 ** (np.arange(0, HD, 2, dtype=np.float32) / HD))
    t = np.arange(S, dtype=np.float32)
    f = np.outer(t, inv).astype(np.float32)  # [S, HD//2]
    return np.ascontiguousarray(np.cos(f).T), np.ascontiguousarray(np.sin(f).T)


def causal_masks(TC):
    # masks[dd][k, qrel] = 1 if k + dd*128 <= qrel else 0
    out = np.zeros((4 * 128, TC), BF16)
    k = np.arange(128)[:, None]
    q = np.arange(TC)[None, :]
    for dd in range(4):
        out[dd * 128:(dd + 1) * 128] = (k + dd * 128 <= q).astype(BF16)
    return out


def rope_perm(HD):
    # new row i (i < HD//2) = old 2i; new row HD//2+i = old 2i+1
    return np.concatenate([np.arange(0, HD, 2), np.arange(1, HD, 2)])


def make_in_maps(x, wq, wk, wv, wo, *, n_batch_shards, n_head_shards,
                 NQ_TOT, NKV_TOT, HD, TC):
    """Returns list of in_maps, one per core (batch-major: core = b*G + g)."""
    B, S, D = x.shape
    G = n_head_shards
    NQ = NQ_TOT // G
    NKV = NKV_TOT // G
    perm = rope_perm(HD)
    cosT, sinT = rope_tables(S, HD)
    csT = np.concatenate([cosT, sinT], axis=0).astype(BF16)  # [HD, S]
    masks = causal_masks(TC)

    # Per-batch xT (shared across head shards)
    xtb = {}
    for b in range(B):
        xtb[b] = np.ascontiguousarray(x[b].T).astype(BF16)  # [D, S]

    # Per-headgroup weight shards
    wshard = {}
    for g in range(G):
        qrows = slice(g * NQ * HD, (g + 1) * NQ * HD)
        kvrows = slice(g * NKV * HD, (g + 1) * NKV * HD)
        wq_g = wq[qrows, :].copy()      # [NQ*HD, D]
        wk_g = wk[kvrows, :].copy()
        wv_g = wv[kvrows, :].copy()
        # RoPE permutation of output rows, per head
        for hh in range(NQ):
            blk = wq_g[hh * HD:(hh + 1) * HD]
            wq_g[hh * HD:(hh + 1) * HD] = blk[perm]
        for hh in range(NKV):
            blk = wk_g[hh * HD:(hh + 1) * HD]
            wk_g[hh * HD:(hh + 1) * HD] = blk[perm]
        wqT = np.ascontiguousarray(wq_g.T).astype(BF16)   # [D, NQ*HD]
        wkT = np.ascontiguousarray(wk_g.T).astype(BF16)
        wvT = np.ascontiguousarray(wv_g.T).astype(BF16)
        woT = np.ascontiguousarray(wo[:, qrows].T).astype(BF16)  # [NQ*HD, D]
        wshard[g] = (wqT, wkT, wvT, woT)

    ident = np.eye(128, dtype=BF16)
    in_maps = []
    for b in range(n_batch_shards):
        for g in range(G):
            wqT, wkT, wvT, woT = wshard[g]
            in_maps.append({
                "xt": xtb[b],
                "wqp": wqT, "wkp": wkT, "wvp": wvT, "woh": woT,
                "csT": csT,
                "masks": masks,
                "ident": ident,
            })
    return in_maps


def combine_outputs(outTs, B, G):
    """outTs: list of [D, S] partials, core order b*G+g. Returns [B, S, D]."""
    outs = []
    for b in range(B):
        acc = outTs[b * G].astype(np.float32).copy()
        for g in range(1, G):
            acc += outTs[b * G + g]
        outs.append(acc.T)  # [S, D]
    return np.stack(outs)


_NC_CACHE = {}


def _get_nc(S, D, NQ, NKV, HD, TC):
    key = (S, D, NQ, NKV, HD, TC)
    if key not in _NC_CACHE:
        nc = build_attention_nc(S=S, D=D, NQ=NQ, NKV=NKV, HD=HD, TC=TC)
        split_multiwait_insts(nc)
        _NC_CACHE[key] = nc
    return _NC_CACHE[key]


def kernel(**inputs):
    x = np.asarray(inputs["x"], dtype=np.float32)
    wq = np.asarray(inputs["wq"], dtype=np.float32)
    wk = np.asarray(inputs["wk"], dtype=np.float32)
    wv = np.asarray(inputs["wv"], dtype=np.float32)
    wo = np.asarray(inputs["wo"], dtype=np.float32)

    B, S, D = x.shape          # (2, 2048, 2048)
    NQ_TOT = wq.shape[0] // 128
    NKV_TOT = wk.shape[0] // 128
    HD = 128
    TC = 512
    G = 4                      # head shards
    NQ, NKV = NQ_TOT // G, NKV_TOT // G

    nc = _get_nc(S, D, NQ, NKV, HD, TC)
    in_maps = make_in_maps(
        x, wq, wk, wv, wo,
        n_batch_shards=B, n_head_shards=G,
        NQ_TOT=NQ_TOT, NKV_TOT=NKV_TOT, HD=HD, TC=TC,
    )

    from concourse.bass_utils import run_bass_kernel_spmd

    trace = os.environ.get("BASS_ATTN_TRACE") == "1"
    res = run_bass_kernel_spmd(nc, in_maps, list(range(len(in_maps))), trace=trace)
    kernel.last_results = res
    outTs = [r["outT"] for r in res.results]
    return combine_outputs(outTs, B, G).astype(np.float32)
